# revision 40
# baseline (speedup 1.0000x reference)
"""Trainium2 Bass kernel for nn_AttentionLayer (sparse euclidean attention).

Math (reference):
    a      = tanh(attended @ W_A_X) + b_A_X          [L, D]
    M[i,j] = sum_d W_A[d] * (a[j,d] - a[i,d])^2      (>=0, 0 on diagonal)
    energy = softmax(-M, axis=1)
    glimpsed = energy @ source
    out    = tanh(concat([glimpsed, source]) @ W_A_combine) + b_A_combine

Rewrite used here: with b = a * W_A and wsq[j] = a[j]·b[j],
    -M[i,j] = 2*a_i·b_j - wsq_i - wsq_j.
wsq_i is constant per row and cancels in the softmax, so per query q:
    E'[k,q]   = exp(2*(a_q·b_k) - wsq_k - CSHIFT)
    energy    = E' / sum_k E'
No row-max pass is needed: the true max logit sits at k=q (M=0), and with
CSHIFT=40 every quantity stays comfortably inside fp32 (wsq is ~[36, 61]
for the target distribution; safe for wsq up to ~125).

Distribution: queries (rows) sharded 8 ways; every core holds the full
key-side tensors (b^T, wsq, source) which it computes itself from the
replicated attended^T input. No collectives.

Layouts (per core, transposed so no on-chip transposes are ever needed):
    mm1:  S^T[k,q]  = bT[d,k].T @ aqT[d,q]     (PSUM [128k, 512q])
    exp:  E'^T[k,q] = Exp(2*S^T + bias[k])     (ACT, per-partition bias)
    mm2:  G^T[s,q] += srcN[k,s].T @ E'^T[k,q]  (accumulate over 64 k-tiles)
    den:  den_acc[128,q] += E'^T  on DVE; one ones-matmul at the end
          replicates the partition-sum, reciprocal_approx_fast inverts.
    comb: out^T[s',q] = tanh(Wc[c,s'].T @ [G^T/den ; srcT_q]) + b_c

v4 notes:
- Engine queues are FIFO in emission order, so the prologue and main
  loop are software-pipelined at EMISSION level: l-tiles 8-15 are
  interleaved with main-loop k-tiles 0-23 of q-tile 0 (3 per l-tile).
  All cross-phase tensors (attT, bT, srcN, wsqT, negrow) are chunked
  into separate tiles so dependencies resolve per-chunk.
- den removed from PE (was 64x 307ns [128,1,512] matmuls per q-tile)
  and accumulated on DVE (gpsimd measured 3x slower than DVE for
  [128,512] tensor ops - keep it to memsets/DMA triggers only).
- wsq row->bias transpose via one DRAM round trip (512B-contiguous
  descriptors) + PE transpose per half, not a 16K 4B-descriptor
  scatter.
- All DRAM inputs host-prepped partition-major; a few big DMAs split
  between the two HWDGE rings (sync + scalar) in priority order.
"""

import numpy as np

L = 8192
D = 256
S = 256
NCORES = 8
Q = L // NCORES          # 1024 queries per core
KT = 128                 # key tile (PSUM partition dim)
NK = L // KT             # 64 key tiles
LT = 512                 # prologue l-tile width
NL = L // LT             # 16 prologue tiles
QT = 512                 # query tile (PSUM free dim)
NQ = Q // QT             # 2 query tiles per core
CSHIFT = 40.0
DELAY = 2                # k-tiles mm1/exp run ahead of mm2
PRO_STEPS = 2            # main-loop k-tiles interleaved per late l-tile

_cache = {}


def _build():
    import concourse.bass as bass
    import concourse.tile as tile
    from concourse import bacc, mybir, masks

    F32 = mybir.dt.float32
    BF16 = mybir.dt.bfloat16
    AF = mybir.ActivationFunctionType
    ALU = mybir.AluOpType

    nc = bacc.Bacc("TRN2", target_bir_lowering=False, debug=False)

    # host-prepped, partition-major DRAM inputs
    attT_d = nc.dram_tensor("attT", [4, 128, 2, 2048], BF16, kind="ExternalInput")
    attTq_d = nc.dram_tensor("attTq", [128, 2, Q], BF16, kind="ExternalInput")
    srcN_d = nc.dram_tensor("srcN", [2, 128, 32, S], BF16, kind="ExternalInput")
    srcTq_d = nc.dram_tensor("srcTq", [128, 2, Q], BF16, kind="ExternalInput")
    waxT_d = nc.dram_tensor("waxT", [128, 2, 2, 128], BF16, kind="ExternalInput")
    wc_d = nc.dram_tensor("wc", [128, 4, 2, 128], BF16, kind="ExternalInput")
    smalls_d = nc.dram_tensor("smalls", [128, 8], F32, kind="ExternalInput")
    out_d = nc.dram_tensor("out", [2, 128, Q], F32, kind="ExternalOutput")

    with tile.TileContext(nc) as tc:
        with tc.tile_pool(name="persist", bufs=1) as persist:
            # chunked cross-phase tensors (separate tiles => per-chunk deps)
            bT = [persist.tile([128, 2, 2048], BF16, tag=f"bT{i}", name=f"bT{i}")
                  for i in range(4)]
            attT_sb = [persist.tile([128, 2, 2048], BF16, tag=f"attT{i}",
                                    name=f"attT{i}") for i in range(4)]
            srcN_sb = [persist.tile([128, 32, S], BF16, tag=f"srcN{i}",
                                    name=f"srcN{i}") for i in range(2)]
            negrow = [persist.tile([1, 4096], F32, tag=f"negrow{i}",
                                   name=f"negrow{i}") for i in range(2)]
            wsq64 = [persist.tile([32, 128], F32, tag=f"wsq64_{i}",
                                  name=f"wsq64_{i}") for i in range(2)]
            wsqT = [persist.tile([128, 32], F32, tag=f"wsqT{i}",
                                 name=f"wsqT{i}") for i in range(2)]
            aq = persist.tile([128, 2, Q], BF16, tag="aq")
            srcTq_sb = persist.tile([128, 2, Q], BF16, tag="srcTq")
            attTq_sb = persist.tile([128, 2, Q], BF16, tag="attTq")
            waxT_sb = persist.tile([128, 2, 2, 128], BF16, tag="waxT")
            wc_sb = persist.tile([128, 4, 2, 128], BF16, tag="wc")
            smalls_sb = persist.tile([128, 8], F32, tag="smalls")
            negones_sb = persist.tile([128, 1], BF16, tag="negones")
            ones_sb = persist.tile([128, 128], BF16, tag="ones")
            ident_sb = persist.tile([32, 32], F32, tag="ident")

            bax = [smalls_sb[:, c:c + 1] for c in range(2)]
            swa = [smalls_sb[:, 2 + c:3 + c] for c in range(2)]   # sqrt(wa)
            bac = [smalls_sb[:, 4 + c:5 + c] for c in range(2)]
            swab = [smalls_sb[:, 6 + c:7 + c] for c in range(2)]  # sqrt(wa)*bax

            # --- input DMAs: ALL bulk on the sync ring in consumption
            # order (srcN after attT so attT chunks never wait behind it);
            # scalar ring takes the small tensors + later the wsq round
            # trips ---
            nc.sync.dma_start(
                out=attT_sb[0][:, :, 0:1024], in_=attT_d[0, :, :, 0:1024]
            )
            nc.sync.dma_start(out=waxT_sb[:], in_=waxT_d[:])
            nc.scalar.dma_start(out=smalls_sb[:], in_=smalls_d[:])
            nc.sync.dma_start(
                out=attT_sb[0][:, :, 1024:2048], in_=attT_d[0, :, :, 1024:2048]
            )
            nc.sync.dma_start(out=attTq_sb[:], in_=attTq_d[:])
            for t in range(1, 4):
                for hh in range(2):
                    nc.sync.dma_start(
                        out=attT_sb[t][:, :, hh * 1024:(hh + 1) * 1024],
                        in_=attT_d[t, :, :, hh * 1024:(hh + 1) * 1024],
                    )
            nc.scalar.dma_start(out=srcTq_sb[:], in_=srcTq_d[:])
            nc.scalar.dma_start(out=wc_sb[:], in_=wc_d[:])
            for hh in range(2):
                nc.sync.dma_start(out=srcN_sb[hh][:], in_=srcN_d[hh])

            nc.vector.memset(ones_sb[:], 1.0)
            nc.vector.memset(negones_sb[:], -1.0)
            masks.make_identity(nc, ident_sb[:])

            with tc.tile_pool(name="dr", bufs=1, space="DRAM") as dr, \
                 tc.tile_pool(name="atq", bufs=2) as atq_p, \
                 tc.tile_pool(name="at", bufs=3) as at_p, \
                 tc.tile_pool(name="sq", bufs=3) as sq_p, \
                 tc.tile_pool(name="eT", bufs=9) as eT_p, \
                 tc.tile_pool(name="gN", bufs=2) as gN_p, \
                 tc.tile_pool(name="ct", bufs=2) as ct_p, \
                 tc.tile_pool(name="rcp", bufs=2) as rcp_p, \
                 tc.tile_pool(name="dacc", bufs=2) as dacc_p, \
                 tc.tile_pool(name="psS", bufs=5, space="PSUM") as psS, \
                 tc.tile_pool(name="psG", bufs=1, space="PSUM") as psG, \
                 tc.tile_pool(name="psD", bufs=1, space="PSUM") as psD:

                wsq_dram = dr.tile([L], F32, tag="wsq_dram")

                # ============ query transform: aq = a^T[:, own] ============
                def emit_queries():
                    for h in range(NQ):
                        for m in range(2):
                            ps = psS.tile([128, QT], F32, tag="s")
                            for c in range(2):
                                nc.tensor.matmul(
                                    ps[:],
                                    waxT_sb[:, c, m, :],
                                    attTq_sb[:, c, h * QT:(h + 1) * QT],
                                    start=(c == 0), stop=(c == 1),
                                )
                            at_q = atq_p.tile([128, QT], F32, tag="atq")
                            nc.scalar.activation(
                                out=at_q[:], in_=ps[:], func=AF.Tanh
                            )
                            nc.vector.tensor_scalar(
                                aq[:, m, h * QT:(h + 1) * QT], at_q[:],
                                bax[m], swa[m],
                                op0=ALU.add, op1=ALU.mult,
                            )

                # ========== prologue l-tile bodies ==========
                # per l-tile: mma -> tanh(ACT) ->
                #   bT_c = tanh*wa + (wa*bax)   (DVE fused tensor_scalar)
                #   sq_0 = (tanh+bax)^2         (ACT Square with bias)
                #   sq_1 = (tanh+bax)^2         (DVE add, DVE mult)
                #   psW  = -sum_d W_A*sq        (PE rank-1, both chunks)
                mma_state = {}

                def emit_mma(t):
                    pss = []
                    for m in range(2):
                        ps = psS.tile([128, LT], F32, tag="s")
                        for c in range(2):
                            nc.tensor.matmul(
                                ps[:],
                                waxT_sb[:, c, m, :],
                                attT_sb[t // 4][:, c, (t % 4) * LT:(t % 4 + 1) * LT],
                                start=(c == 0), stop=(c == 1),
                            )
                        pss.append(ps)
                    return pss

                def emit_ltile(t):
                    # bT' = sqrt(wa)*(tanh+bax) on DVE (c0) / ACT-Identity
                    # (c1); sq = bT'*bT' in one DVE 16-bit op; the wsq
                    # reduction is then a plain -ones matmul.
                    ps_prev = mma_state.pop(t)
                    at_t = at_p.tile([128, 2, LT], F32, tag="at")
                    for m in range(2):
                        nc.scalar.activation(
                            out=at_t[:, m, :], in_=ps_prev[m][:], func=AF.Tanh,
                        )
                    sq_t = sq_p.tile([128, 2, LT], BF16, tag="sq")
                    cc = t // 4
                    ll = (t % 4) * LT
                    nc.vector.tensor_scalar(
                        bT[cc][:, 0, ll:ll + LT], at_t[:, 0, :],
                        bax[0], swa[0],
                        op0=ALU.add, op1=ALU.mult,
                    )
                    nc.gpsimd.tensor_scalar(
                        bT[cc][:, 1, ll:ll + LT], at_t[:, 1, :],
                        bax[1], swa[1],
                        op0=ALU.add, op1=ALU.mult,
                    )
                    nc.vector.tensor_tensor(
                        out=sq_t[:], in0=bT[cc][:, :, ll:ll + LT],
                        in1=bT[cc][:, :, ll:ll + LT], op=ALU.mult,
                    )
                    ps_w = psS.tile([1, LT], F32, tag="s")
                    for c in range(2):
                        nc.tensor.matmul(
                            ps_w[:], negones_sb[:], sq_t[:, c, :],
                            start=(c == 0), stop=(c == 1),
                        )
                    nc.vector.tensor_copy(
                        out=negrow[t // 8][0:1, (t % 8) * LT:(t % 8 + 1) * LT],
                        in_=ps_w[:],
                    )
                    if t + 1 < NL:
                        mma_state[t + 1] = emit_mma(t + 1)

                def wsq_dmas(hh, eng):
                    # negrow holds -wsq; round-trip through DRAM to get it
                    # partition-distributed (HWDGE ring, 512B-contiguous
                    # descriptors both ways)
                    eng.dma_start(
                        out=wsq_dram[hh * 4096:(hh + 1) * 4096],
                        in_=negrow[hh][0:1, :],
                    )
                    eng.dma_start(
                        out=wsq64[hh][:],
                        in_=bass.AP(
                            tensor=wsq_dram.tensor,
                            offset=wsq_dram.offset + hh * 4096,
                            ap=[[128, 32], [1, 128]],
                        ),
                    )

                def wsq_transpose(hh):
                    # PE-transpose [32,128] -> bias layout [128, 32]; emitted
                    # a couple of l-tiles after wsq_dmas so the PE never
                    # stalls on the DMA round trip.
                    ps_t = psS.tile([128, 32], F32, tag="s")
                    nc.tensor.transpose(ps_t[:], wsq64[hh][:], ident_sb[:])
                    nc.vector.tensor_scalar_add(wsqT[hh][:], ps_t[:], -CSHIFT)

                # ========== main-loop q-tile emission (step-driven) ==========
                def make_qtile(h, final_tail=False):
                    aq0 = aq[:, 0, h * QT:(h + 1) * QT]
                    aq1 = aq[:, 1, h * QT:(h + 1) * QT]
                    ps_g = psG.tile([128, 2, QT], F32, tag="psG")
                    den = dacc_p.tile([128, QT], F32, tag="dacc")
                    st = {"ss": [], "es": [], "t": 0}

                    def emit_mm1(t):
                        ps_s = psS.tile([128, QT], F32, tag="s")
                        cc, kk = t // 16, (t % 16) * KT
                        nc.tensor.matmul(
                            ps_s[:], bT[cc][:, 0, kk:kk + KT], aq0,
                            start=True, stop=False,
                        )
                        nc.tensor.matmul(
                            ps_s[:], bT[cc][:, 1, kk:kk + KT], aq1,
                            start=False, stop=True,
                        )
                        return ps_s

                    def emit_exp(t):
                        e_t = eT_p.tile([128, QT], BF16, tag="eT")
                        nc.scalar.activation(
                            out=e_t[:], in_=st["ss"][t][:], func=AF.Exp,
                            bias=wsqT[t // 32][:, t % 32:t % 32 + 1],
                            scale=2.0,
                        )
                        return e_t

                    def prime():
                        st["ss"].append(emit_mm1(0))
                        for t in range(DELAY):
                            st["es"].append(emit_exp(t))
                            st["ss"].append(emit_mm1(t + 1))

                    def step():
                        t = st["t"]
                        st["t"] = t + 1
                        es = st["es"]
                        if t + DELAY < NK:
                            es.append(emit_exp(t + DELAY))
                            st["ss"].append(
                                emit_mm1(t + DELAY + 1) if t + DELAY + 1 < NK
                                else None
                            )
                        for m in range(2):
                            nc.tensor.matmul(
                                ps_g[:, m, :],
                                srcN_sb[t // 32][:, t % 32, m * 128:(m + 1) * 128],
                                es[t][:],
                                start=(t == 0), stop=(t == NK - 1),
                            )
                        if t == 1:
                            nc.vector.tensor_tensor(
                                out=den[:], in0=es[0][:], in1=es[1][:],
                                op=ALU.add,
                            )
                        elif t > 1:
                            nc.vector.tensor_tensor(
                                out=den[:], in0=den[:], in1=es[t][:],
                                op=ALU.add,
                            )

                    def tail():
                        # In the final tail the srcTq halves of both combine
                        # groups are emitted first: they keep the PE busy
                        # while the den broadcast/reciprocal chain resolves.
                        # (Not done for tail0 - holding 2 psS slots that
                        # long would stall qtile1's mm1 ring.)
                        den_bf = rcp_p.tile([128, QT], BF16, tag="denbf")
                        nc.vector.tensor_copy(out=den_bf[:], in_=den[:])
                        ps_cs = []
                        if final_tail:
                            for m in range(2):
                                ps_c = psS.tile([128, QT], F32, tag="s")
                                for c in range(2):
                                    nc.tensor.matmul(
                                        ps_c[:], wc_sb[:, 2 + c, m, :],
                                        srcTq_sb[:, c, h * QT:(h + 1) * QT],
                                        start=(c == 0), stop=False,
                                    )
                                ps_cs.append(ps_c)
                        ps_d = psD.tile([128, QT], F32, tag="psD")
                        nc.tensor.matmul(
                            ps_d[:], ones_sb[:], den_bf[:],
                            start=True, stop=True,
                        )
                        rcp = rcp_p.tile([128, QT], F32, tag="rcp")
                        nc.vector.reciprocal_approx_fast(out=rcp[:], in_=ps_d[:])
                        g_n = gN_p.tile([128, 2, QT], BF16, tag="gN")
                        for m in range(2):
                            nc.vector.tensor_tensor(
                                out=g_n[:, m, :], in0=ps_g[:, m, :],
                                in1=rcp[:], op=ALU.mult,
                            )
                        for m in range(2):
                            if final_tail:
                                ps_c = ps_cs[m]
                                for c in range(2):
                                    nc.tensor.matmul(
                                        ps_c[:], wc_sb[:, c, m, :],
                                        g_n[:, c, :],
                                        start=False, stop=(c == 1),
                                    )
                            else:
                                ps_c = psS.tile([128, QT], F32, tag="s")
                                rhss = [
                                    srcTq_sb[:, 0, h * QT:(h + 1) * QT],
                                    srcTq_sb[:, 1, h * QT:(h + 1) * QT],
                                    g_n[:, 0, :], g_n[:, 1, :],
                                ]
                                cidx = [2, 3, 0, 1]
                                for j in range(4):
                                    nc.tensor.matmul(
                                        ps_c[:], wc_sb[:, cidx[j], m, :],
                                        rhss[j],
                                        start=(j == 0), stop=(j == 3),
                                    )
                            c_t = ct_p.tile([128, QT], F32, tag="ct")
                            nc.scalar.activation(
                                out=c_t[:], in_=ps_c[:], func=AF.Tanh
                            )
                            nc.vector.tensor_scalar_add(c_t[:], c_t[:], bac[m])
                            (nc.sync if m == 0 else nc.scalar).dma_start(
                                out=out_d[m, :, h * QT:(h + 1) * QT],
                                in_=c_t[:],
                            )
                    return prime, step, tail

                # ========== emission schedule ==========
                # phase 1: l-tiles 0-9 (sq1 alternates ACT/DVE); wsq half-0
                #   DMA round trip issued after l7, its PE transpose two
                #   l-tiles later so the PE never waits on it.
                # phase 2: l-tiles 10-15 interleaved with k-tiles 0-23 of
                #   q-tile 0 (4 per l-tile; exp lookahead DELAY=2 stays
                #   inside wsq half 0).
                # phase 3: wsq half-1 round trip covered by k-tiles 24-29,
                #   then the rest of q-tile 0 and all of q-tile 1.
                mma_state[0] = emit_mma(0)
                for t in range(8):
                    emit_ltile(t)
                wsq_dmas(0, nc.scalar)
                emit_queries()
                emit_ltile(8)
                emit_ltile(9)
                wsq_transpose(0)

                prime0, step0, tail0 = make_qtile(0)
                prime0()
                for t in range(10, NL):
                    emit_ltile(t)
                    for _ in range(PRO_STEPS):
                        step0()
                wsq_dmas(1, nc.scalar)
                for _ in range(6):
                    step0()
                wsq_transpose(1)
                for _ in range(NK - PRO_STEPS * 6 - 6):
                    step0()

                # qtile1's mm1 pipeline primes before tail0 so the PE keeps
                # streaming; tail0 must precede qtile1's first mm2 (psG WAR).
                prime1, step1, tail1 = make_qtile(1, final_tail=True)
                prime1()
                tail0()
                for _ in range(NK):
                    step1()
                tail1()

    nc.compile()
    return nc


def _get_nc():
    if "nc" not in _cache:
        _cache["nc"] = _build()
    return _cache["nc"]


def _bf16(x):
    import ml_dtypes

    return np.ascontiguousarray(x, dtype=ml_dtypes.bfloat16)


def _prep_inputs(attended, source, W_A_X, b_A_X, W_A, W_A_combine, b_A_combine):
    f = np.float32
    att = np.asarray(attended, dtype=f)
    src = np.asarray(source, dtype=f)
    attT = _bf16(att.T.reshape(2, 128, 4, 2048).transpose(2, 1, 0, 3))
    srcN = _bf16(src.reshape(2, 32, 128, S).transpose(0, 2, 1, 3))
    waxT = _bf16(
        np.asarray(W_A_X, dtype=f).reshape(2, 128, 2, 128).transpose(1, 0, 2, 3)
    )
    wc = _bf16(
        np.asarray(W_A_combine, dtype=f).reshape(4, 128, 2, 128).transpose(1, 0, 2, 3)
    )
    bax = np.asarray(b_A_X, dtype=f)
    wa = np.asarray(W_A, dtype=f)
    swa = np.sqrt(wa)
    bac = np.asarray(b_A_combine, dtype=f)
    smalls = np.ascontiguousarray(
        np.stack(
            [bax[:128], bax[128:], swa[:128], swa[128:], bac[:128], bac[128:],
             swa[:128] * bax[:128], swa[128:] * bax[128:]],
            axis=1,
        )
    )

    in_maps = []
    for i in range(NCORES):
        sl = slice(i * Q, (i + 1) * Q)
        attTq = _bf16(att[sl].T.reshape(2, 128, Q).transpose(1, 0, 2))
        srcTq = _bf16(src[sl].T.reshape(2, 128, Q).transpose(1, 0, 2))
        in_maps.append({
            "attT": attT, "attTq": attTq, "srcN": srcN, "srcTq": srcTq,
            "waxT": waxT, "wc": wc, "smalls": smalls,
        })
    return in_maps


def _run(in_maps, trace=False):
    from concourse.bass_utils import run_bass_kernel_spmd

    nc = _get_nc()
    res = run_bass_kernel_spmd(nc, in_maps, list(range(NCORES)), trace=trace)
    _cache["last_result"] = res
    out = np.empty((L, S), dtype=np.float32)
    for i in range(NCORES):
        o = res.results[i]["out"]          # [2, 128, Q] = combined^T chunks
        out[i * Q:(i + 1) * Q, :] = np.asarray(o, dtype=np.float32).reshape(S, Q).T
    return out


def kernel(**inputs):
    in_maps = _prep_inputs(**inputs)
    return _run(in_maps, trace=False)
